# revision 41
# baseline (speedup 1.0000x reference)
"""Trainium2 Bass kernel for nn_MultiHeadAttention (dense transformer block).

Reference computation (B=2 batches, N=2048 tokens, C=1024, H=16 heads, D=64):
    qkv  = x @ W_qkv.T + b_qkv
    q,k,v split into heads; attn = softmax(q @ k.T / sqrt(D)); o = attn @ v
    out  = o @ W_proj.T + b_proj

Sharding over 8 NeuronCores: head-parallel attention, token-parallel
projection.  Core c owns heads {2c, 2c+1} and computes QKV + attention for
both batches for those heads.  The per-head outputs o^T are exchanged with a
single 8-way AllToAll per head (each core sends, for every peer r, its head's
o^T slice for peer r's (batch, token-slice)); afterwards each core holds
o^T of ALL 16 heads for its own 512-token slice (batch c//4, tokens
(c%4)*512..) and computes the full output projection there.

All matmuls run in bf16 (fp32 PSUM accumulation).  attn@V uses the exp tile
as the stationary operand and V (with an appended ones column) as the moving
operand, so each matmul streams only 65 rows at full 128x128 PE utilization
and the softmax denominator lands on the same PSUM partition as the outputs
(normalization = per-partition scalar multiply on the vector engine).
"""

import sys

sys.path.insert(0, "/opt/trn_rl_repo")

import numpy as np
import ml_dtypes
import concourse.bass as bass
import concourse.tile as tile
from concourse import mybir, bacc
from concourse.bass_utils import run_bass_kernel_spmd

f32 = mybir.dt.float32
bf16 = mybir.dt.bfloat16
f8 = mybir.dt.float8e4
np_bf16 = ml_dtypes.bfloat16

# problem constants (hardcoded per contract)
B = 2
N = 2048
C = 1024
H = 16
D = C // H  # 64
SCALE = D ** -0.5

NCORES = 8
GROUPS8 = [[0, 1, 2, 3, 4, 5, 6, 7]]
HPC = H // NCORES          # heads per core = 2
TOKS = N // 4              # per-core output token slice = 512
N_CT = C // 128            # contraction chunks over C = 8
N_JT = N // 128            # key tiles = 16
N_IC = N // 512            # query blocks = 4
N_OD = C // 128            # o-dim contraction chunks in proj = 8


def build_kernel():
    nc = bacc.Bacc("TRN2", target_bir_lowering=False, debug=False,
                   num_devices=NCORES)

    # ---- DRAM I/O (all bf16 except biases / final output) ----
    xt = nc.dram_tensor("xt", [B, C, N], bf16, kind="ExternalInput").ap()
    # fused [k|q|v] weight slab: [ct, 128 part, 3*128] -> single DMA
    wkqv = nc.dram_tensor("wkqv", [N_CT, 128, 3 * 128], bf16,
                          kind="ExternalInput").ap()
    bqk = nc.dram_tensor("bqk", [128, 2], f32, kind="ExternalInput").ap()
    bv = nc.dram_tensor("bv", [128], bf16, kind="ExternalInput").ap()
    wp_t = nc.dram_tensor("wp_t", [C, C], bf16, kind="ExternalInput").ap()
    bp = nc.dram_tensor("bp", [C], f32, kind="ExternalInput").ap()
    y = nc.dram_tensor("y", [TOKS, C], bf16, kind="ExternalOutput").ap()

    with tile.TileContext(nc, pool_alloc_mode="queue") as tc:
        with (
            tc.tile_pool(name="consts", bufs=1) as consts,
            tc.tile_pool(name="persist", bufs=1) as persist,
            tc.tile_pool(name="p1x", bufs=1) as p1x,
            tc.tile_pool(name="p1w", bufs=1) as p1w,
            tc.tile_pool(name="epool", bufs=18) as epool,
            tc.tile_pool(name="opool", bufs=6) as opool,
            tc.tile_pool(name="rpool", bufs=4) as rpool,
            tc.tile_pool(name="ypool", bufs=3) as ypool,
            tc.tile_pool(name="ps_s", bufs=2, space="PSUM") as ps_s_pool,
            tc.tile_pool(name="ps_o", bufs=1, space="PSUM") as ps_o_pool,
            tc.tile_pool(name="misc", bufs=3, space="PSUM") as misc,
            tc.tile_pool(name="dram", bufs=1, space="DRAM") as dram,
        ):
            # ---------------- constants ----------------
            bqk_sb = consts.tile([128, 2], f32)
            nc.sync.dma_start(out=bqk_sb, in_=bqk)
            # V bias broadcast: [128 part, jt-dup 2, h 2, d 64]
            bv_bc = consts.tile([128, 2, 2, 64], bf16)
            nc.sync.dma_start(
                out=bv_bc,
                in_=bass.AP(tensor=bv.tensor, offset=bv.offset,
                            ap=[[0, 128], [0, 2], [64, 2], [1, 64]]),
            )
            bp_bc = consts.tile([128, C], f32)
            # exp shift (keeps fp8 exp in range; cancels in softmax ratio)
            eshift = consts.tile([128, 1], f32)
            nc.vector.memset(eshift, -4.0)
            # 128x128 bf16 identity (moving operand of PE transposes)
            ident = consts.tile([128, 128], bf16)
            nc.gpsimd.memset(ident, 0.0)
            nc.gpsimd.affine_select(
                out=ident, in_=ident,
                compare_op=mybir.AluOpType.not_equal,
                fill=1.0, base=0, pattern=[[-1, 128]], channel_multiplier=1,
            )

            # -------------- persistent activations --------------
            # partition dim = 2 local heads x 64 dims (bf16: fp8 attention
            # was tested and fails the 2e-2 tolerance - the near-diagonal
            # logits reach 9.0, so rows are peaked and quantization noise
            # on q/k/e/v transfers directly into the output)
            qt_sb = persist.tile([128, B, N], bf16)   # q^T
            kt_sb = persist.tile([128, B, N], bf16)   # k^T
            # V natural + ones column: [tok-part, b, jt, h, 65]
            vp_sb = persist.tile([128, B, N_JT, HPC, 65], bf16)
            nc.vector.memset(vp_sb[:, :, :, :, 64:65], 1.0)
            # o^T (normalized): [64 dims, head, b, t] - 64-partition tile so
            # all engine copies into it stay partition-base aligned
            ot_sb = persist.tile([64, HPC, B, N], bf16)

            # weight / x staging
            xt_sb = p1x.tile([128, N_CT, N], bf16)
            wkqv_sb = p1w.tile([128, N_CT, 3 * 128], bf16)
            wk_sb = wkqv_sb[:, :, 0:128]
            wq_sb = wkqv_sb[:, :, 128:256]
            wv_sb = wkqv_sb[:, :, 256:384]
            wp_sb = p1w.tile([128, N_OD, C], bf16)
            ogt_sb = persist.tile([128, N_OD, TOKS], bf16)

            xt_views = [
                xt[b].rearrange("(ct p) n -> p ct n", p=128) for b in range(B)
            ]

            def emit_xt_dma(b):
                for ct in range(N_CT):
                    nc.sync.dma_start(out=xt_sb[:, ct, :],
                                      in_=xt_views[b][:, ct, :])

            # W_kqv (one instruction) + x^T(b0); W_p deferred until the
            # attention phase is underway (it is first read ~150us in).
            nc.sync.dma_start(out=wkqv_sb,
                              in_=wkqv.rearrange("ct p m -> p ct m"))
            emit_xt_dma(0)

            def emit_qk(b, tensor, segs):
                """q^T / k^T projection for 512-token segments `segs`."""
                w_sb = wq_sb if tensor == "q" else wk_sb
                bcol = 0 if tensor == "q" else 1
                for seg in segs:
                    ps = misc.tile([128, 512], f32, tag="mpsum")
                    for ct in range(N_CT):
                        nc.tensor.matmul(
                            ps,
                            lhsT=w_sb[:, ct, :],
                            rhs=xt_sb[:, ct, seg * 512:(seg + 1) * 512],
                            start=(ct == 0), stop=(ct == N_CT - 1),
                        )
                    sl = slice(seg * 512, (seg + 1) * 512)
                    dst = qt_sb[:, b, sl] if tensor == "q" else kt_sb[:, b, sl]
                    nc.vector.tensor_scalar_add(
                        out=dst,
                        in0=ps,
                        scalar1=bqk_sb[:, bcol:bcol + 1],
                    )

            def emit_v(b, pairs):
                """V natural projection for pairs of 128-token tiles."""
                for p in pairs:
                    ps = misc.tile([128, 2, HPC, 64], f32, tag="mpsum")
                    for g in range(2):
                        tt = 2 * p + g
                        for ct in range(N_CT):
                            nc.tensor.matmul(
                                ps[:, g, :, :],
                                lhsT=xt_sb[:, ct, tt * 128:(tt + 1) * 128],
                                rhs=wv_sb[:, ct, :],
                                start=(ct == 0), stop=(ct == N_CT - 1),
                            )
                    nc.vector.tensor_add(
                        out=vp_sb[:, b, 2 * p:2 * p + 2, :, 0:64],
                        in0=ps,
                        in1=bv_bc,
                    )

            def emit_scores_exp(s, b, ic):
                """scores + exp for head s, batch b, 512-query block ic.
                Returns the 8 exp tiles ([128 keys, 2 jt x 512 q] each)."""
                e_tiles = []
                for jp in range(8):
                    ps = ps_s_pool.tile([128, 1024], f32, tag="ps_s")
                    for j2 in range(2):
                        jt = jp * 2 + j2
                        nc.tensor.matmul(
                            ps[:, j2 * 512:(j2 + 1) * 512],
                            lhsT=kt_sb[64 * s:64 * (s + 1), b,
                                       jt * 128:(jt + 1) * 128],
                            rhs=qt_sb[64 * s:64 * (s + 1), b,
                                      ic * 512:(ic + 1) * 512],
                            start=True, stop=True,
                        )
                    # constant shift (cancels in the softmax ratio) keeps the
                    # largest exp values small; softmax scale applied here in
                    # f32 rather than folded into quantized weights
                    e = epool.tile([128, 1024], bf16, tag="e")
                    nc.scalar.activation(
                        out=e, in_=ps,
                        func=mybir.ActivationFunctionType.Exp,
                        scale=SCALE, bias=eshift[:, 0:1],
                    )
                    e_tiles.append(e)
                return e_tiles

            def emit_attnv(s, b, ic, e_tiles, last=False):
                """attn@V + normalize + transpose into ot_sb.  For the
                final pre-collective block the normalize multiplies run on
                the (by then idle) activation engine, shortening the
                cross-engine latency chain ahead of the exposed AllToAll."""
                ps_o = ps_o_pool.tile([128, 4, 65], f32, tag="ps_o")
                for qt in range(4):
                    for jt in range(N_JT):
                        e = e_tiles[jt // 2]
                        qoff = (jt % 2) * 512 + qt * 128
                        nc.tensor.matmul(
                            ps_o[:, qt, :],
                            lhsT=e[:, qoff:qoff + 128],
                            rhs=vp_sb[:, b, jt, s, :],
                            start=(jt == 0), stop=(jt == N_JT - 1),
                        )
                r = rpool.tile([128, 4, 1], f32, tag="r")
                nc.vector.reciprocal(out=r, in_=ps_o[:, :, 64:65])
                for qt in range(4):
                    o_t = opool.tile([128, 64], bf16, tag="o")
                    if last:
                        nc.scalar.activation(
                            out=o_t, in_=ps_o[:, qt, 0:64],
                            func=mybir.ActivationFunctionType.Copy,
                            scale=r[:, qt, :],
                        )
                    else:
                        nc.vector.tensor_scalar_mul(
                            out=o_t, in0=ps_o[:, qt, 0:64], scalar1=r[:, qt, :],
                        )
                    tp = misc.tile([64, 128], bf16, tag="mpsum")
                    nc.tensor.transpose(tp, in_=o_t, identity=ident)
                    nc.vector.tensor_copy(
                        out=ot_sb[:, s, b,
                                  ic * 512 + qt * 128:ic * 512 + (qt + 1) * 128],
                        in_=tp,
                    )

            # partial projection accumulator (head-1 od chunks, + bias)
            y_acc = persist.tile([128, 4, C], f32)

            at_in = [dram.tile([512, 512], bf16, name=f"at_in{s}")
                     for s in range(HPC)]
            at_out = [dram.tile([512, 512], bf16, name=f"at_out{s}")
                      for s in range(HPC)]

            def emit_at_slice(s, b, ic):
                """ship one completed (head, batch, token-block) o^T slice
                into the AllToAll staging buffer as soon as it exists, so
                only a 64KB slice DMA precedes the final collective."""
                nc.sync.dma_start(
                    out=at_in[s].rearrange("(r p) t -> p r t", p=64)[
                        :, 4 * b + ic, :],
                    in_=ot_sb[:, s, b, ic * 512:(ic + 1) * 512],
                )

            def emit_at(s):
                """8-way AllToAll of head s's o^T + landing DMA into ogt.
                Head 0's landing is chunked per od so the final projection's
                first accumulation chunk starts as early as possible."""
                nc.gpsimd.collective_compute(
                    "AllToAll",
                    mybir.AluOpType.bypass,
                    ins=[at_in[s][:].opt()],
                    outs=[at_out[s][:].opt()],
                    replica_groups=GROUPS8,
                )
                view = at_out[s].rearrange("(o p) t -> p o t", p=128)
                if s == 0:
                    for o in range(4):
                        nc.sync.dma_start(out=ogt_sb[:, o, :],
                                          in_=view[:, o, :])
                else:
                    nc.sync.dma_start(
                        out=ogt_sb[:, s * 4:(s + 1) * 4, :], in_=view)

            def emit_partial_proj(tt):
                """proj over head-1 od chunks (4..7) into y_acc, + bias."""
                for nc2 in range(2):
                    ps = misc.tile([128, 512], f32, tag="mpsum",
                                   name=f"ps_pp{tt}_{nc2}")
                    for i, od in enumerate(range(4, 8)):
                        nc.tensor.matmul(
                            ps,
                            lhsT=ogt_sb[:, od, tt * 128:(tt + 1) * 128],
                            rhs=wp_sb[:, od, nc2 * 512:(nc2 + 1) * 512],
                            start=(i == 0), stop=(i == 3),
                        )
                    nc.vector.tensor_add(
                        out=y_acc[:, tt, nc2 * 512:(nc2 + 1) * 512],
                        in0=ps,
                        in1=bp_bc[:, nc2 * 512:(nc2 + 1) * 512],
                    )

            # ---------------- emission schedule ----------------
            # Unit order: (h0,b0), (h1,b0), (h1,b1), (h0,b1).
            # AT(h1) fires after unit 2 and hides under unit 3; partial
            # projection of the h1 od-chunks runs inside unit 3's blocks;
            # only AT(h0) + the h0 half of proj remain at the end.
            units = [(0, 0), (1, 0), (1, 1), (0, 1)]
            # per-(unit,ic) PE work interleaved into the blocks.  "pre"
            # slots produce data the NEXT block's scores need (q segments)
            # and run before the lookahead scores; "post" slots (v, weight
            # DMAs, ...) run after them, before attnV of the current block.
            # a pre-slot delays the next block's scores+exp, so only the
            # first q segment (needed by the immediately following lookahead)
            # lives there; everything else is emitted post-scores.
            slots_pre = {
                (0, 0): [lambda: emit_qk(0, "q", [1])],
            }
            slots_post = {
                # v(b0) must be fully emitted before unit 0's first attnV.
                # x(b1) load goes after the last emitted x(b0) reader
                # (q(b0) seg3, the (0,2) pre-slot).
                (0, 0): [lambda: emit_qk(0, "q", [2]),
                         lambda: emit_v(0, range(8))],
                (0, 1): [lambda: emit_qk(0, "q", [3])],
                (0, 2): [lambda: emit_xt_dma(1)],
                (0, 3): [lambda: emit_qk(1, "k", [0])],
                (1, 0): [lambda: emit_qk(1, "k", [1]),
                         lambda: nc.sync.dma_start(
                             out=wp_sb,
                             in_=wp_t.rearrange("(od p) c -> p od c", p=128))],
                (1, 1): [lambda: emit_qk(1, "k", [2]),
                         lambda: emit_v(1, [0]),
                         lambda: nc.sync.dma_start(
                             out=bp_bc,
                             in_=bass.AP(tensor=bp.tensor, offset=bp.offset,
                                         ap=[[0, 128]] + bp.ap))],
                (1, 2): [lambda: emit_qk(1, "k", [3]),
                         lambda: emit_qk(1, "q", [0]),
                         lambda: emit_v(1, [1])],
                (1, 3): [lambda: emit_qk(1, "q", [1]),
                         lambda: emit_v(1, [2, 3])],
                (2, 0): [lambda: emit_qk(1, "q", [2]),
                         lambda: emit_v(1, range(4, 8))],
                (2, 1): [lambda: emit_qk(1, "q", [3])],
            }

            # fill warmers: junk matmuls interleaved into the DMA-chased
            # k(b0) projection keep the PE busy-streak alive, so the whole
            # fill and the first attention block dispatch at full p-state
            # instead of the mid-clock ramp (the ps_s pool is idle here)
            wfill = ps_s_pool.tile([128, 1024], f32, tag="ps_s", name="wfill")

            def fill_warm(n):
                for _ in range(n):
                    nc.tensor.matmul(wfill[:, 0:128], lhsT=ident,
                                     rhs=bv_bc[:, 0, :, :],
                                     start=True, stop=True)

            fill_warm(12)
            for seg in range(4):
                ps = misc.tile([128, 512], f32, tag="mpsum")
                for ct in range(N_CT):
                    nc.tensor.matmul(
                        ps,
                        lhsT=wk_sb[:, ct, :],
                        rhs=xt_sb[:, ct, seg * 512:(seg + 1) * 512],
                        start=(ct == 0), stop=(ct == N_CT - 1),
                    )
                    if seg == 0:
                        fill_warm(10)
                nc.vector.tensor_scalar_add(
                    out=kt_sb[:, 0, seg * 512:(seg + 1) * 512],
                    in0=ps,
                    scalar1=bqk_sb[:, 1:2],
                )
            emit_qk(0, "q", [0])

            # flat block list, software-pipelined one block ahead: scores+exp
            # for block j+1 are emitted before attnV of block j, so the
            # activation engine never waits out attnV/transposes at block and
            # unit boundaries.
            blocks = [(ui, s, b, ic)
                      for ui, (s, b) in enumerate(units)
                      for ic in range(N_IC)]
            e_cur = emit_scores_exp(*blocks[0][1:])
            for j, (ui, s, b, ic) in enumerate(blocks):
                for work in slots_pre.get((ui, ic), []):
                    work()
                if j + 1 < len(blocks):
                    e_next = emit_scores_exp(*blocks[j + 1][1:])
                for work in slots_post.get((ui, ic), []):
                    work()
                emit_attnv(s, b, ic, e_cur, last=(j == len(blocks) - 1))
                emit_at_slice(s, b, ic)
                e_cur = e_next
                if ic == N_IC - 1:
                    if ui == 2:
                        emit_at(1)
                    elif ui == 3:
                        emit_at(0)

            # partial projection over the h1 od chunks (landed with AT(1))
            # overlaps AT(0)'s transfer
            for tt in range(TOKS // 128):
                emit_partial_proj(tt)

            # PE warmer: keep the tensor engine clocked up through the AT(0)
            # wait so the final projection issues at full p-state instead of
            # restarting from the low-clock ramp after ~25us of idle.
            warm = misc.tile([128, 512], f32, tag="mpsum", name="warm")
            for _ in range(76):
                nc.tensor.matmul(warm, lhsT=ogt_sb[:, 4, 0:128],
                                 rhs=wp_sb[:, 4, 0:512],
                                 start=True, stop=True)

            # ---------------- final projection (head-0 od chunks) ----------
            # runs after AT(0) lands; uses the attention-phase ps_s pool
            # (free by now) so each token tile needs one psum tile and one
            # wide DVE add
            for tt in range(TOKS // 128):
                ps_y = ps_s_pool.tile([128, 1024], f32, tag="ps_s",
                                      name=f"ps_y{tt}")
                for nc2 in range(2):
                    for od in range(4):
                        nc.tensor.matmul(
                            ps_y[:, nc2 * 512:(nc2 + 1) * 512],
                            lhsT=ogt_sb[:, od, tt * 128:(tt + 1) * 128],
                            rhs=wp_sb[:, od, nc2 * 512:(nc2 + 1) * 512],
                            start=(od == 0), stop=(od == 3),
                        )
                y_sb = ypool.tile([128, C], bf16, tag="y_sb")
                nc.vector.tensor_add(
                    out=y_sb, in0=ps_y, in1=y_acc[:, tt, :],
                )
                nc.sync.dma_start(out=y[tt * 128:(tt + 1) * 128, :], in_=y_sb)

    nc.compile()
    return nc


_CACHE = {}


def _get_nc():
    if "nc" not in _CACHE:
        _CACHE["nc"] = build_kernel()
    return _CACHE["nc"]


def make_in_maps(x, W_qkv, b_qkv, W_proj, b_proj):
    x = np.asarray(x, dtype=np.float32)
    W_qkv = np.asarray(W_qkv, dtype=np.float32)
    b_qkv = np.asarray(b_qkv, dtype=np.float32)
    W_proj = np.asarray(W_proj, dtype=np.float32)
    b_proj = np.asarray(b_proj, dtype=np.float32)

    Wq = W_qkv[0:C]
    Wk = W_qkv[C:2 * C]
    Wv = W_qkv[2 * C:3 * C]
    bq = b_qkv[0:C]
    bk = b_qkv[C:2 * C]
    bv_full = b_qkv[2 * C:3 * C]

    # x^T for both batches, shared by all cores
    xtb = np.ascontiguousarray(
        x.transpose(0, 2, 1)).astype(np_bf16)  # [B, C, N]

    # W_proj.T with rows permuted to the AllToAll arrival order:
    # od chunk (s, o) holds heads {4o+s, 4o+2+s}
    perm = []
    for s in range(HPC):
        for o in range(4):
            for h in (4 * o + s, 4 * o + 2 + s):
                perm.extend(range(64 * h, 64 * (h + 1)))
    wp_t_full = np.ascontiguousarray(W_proj.T[perm, :]).astype(np_bf16)

    in_maps = []
    for core in range(NCORES):
        rows = slice(128 * core, 128 * (core + 1))  # dims of heads 2c, 2c+1
        # fused [k|q|v] weight slab in [ct, 128, 384] layout
        slab = np.concatenate(
            [Wk[rows].T, Wq[rows].T, Wv[rows].T], axis=1)  # [C, 384]
        slab = np.ascontiguousarray(
            slab.reshape(N_CT, 128, 3 * 128)).astype(np_bf16)
        in_maps.append({
            "xt": xtb,
            "wkqv": slab,
            "bqk": np.ascontiguousarray(
                np.stack([bq[rows], bk[rows]], axis=1)),
            "bv": bv_full[rows].astype(np_bf16),
            "wp_t": wp_t_full,
            "bp": b_proj,
        })
    return in_maps


def kernel(x, W_qkv, b_qkv, W_proj, b_proj):
    nc = _get_nc()
    in_maps = make_in_maps(x, W_qkv, b_qkv, W_proj, b_proj)
    res = run_bass_kernel_spmd(nc, in_maps, list(range(NCORES)))

    out = np.empty((B, N, C), dtype=np.float32)
    for core in range(NCORES):
        b = core // 4
        g = core % 4
        out[b, g * TOKS:(g + 1) * TOKS, :] = res.results[core][
            "y"].astype(np.float32)
    return out
